# revision 40
# baseline (speedup 1.0000x reference)
"""Causal self-attention on 8 Trainium2 NeuronCores.

Sharding: 2 heads per core (tensor parallel).  The host pre-transposes the
activations/weights into the layouts the PE array wants, each core computes
QKV -> causal attention -> its partial of the output projection for its two
heads, and the host sums the 8 partial projections (row-parallel linear).

Per-core device program (SPMD, different data per core):
  xT    [1024, 4096]  x transposed, rows=embed c, cols=token t (t = b*2048+tt)
  wqkvT [1024, 384]   w_attn rows for this core's heads, transposed.
                      f = [q_h0 d0..63 | q_h1 | k_h0 | k_h1 | v_h0 | v_h1]
  wpT   [128, 1024]   w_proj columns for this core's channels, transposed
  y     [4096, 1024]  fp16 partial output (sum over cores = final)

Dataflow (everything "transposed" so the PE contraction dim is the partition
dim with no on-device transposes of activations):
  qkvT[f, t]   = wqkvT_tile.T @ xT_tile            (accumulate over 8 c-tiles)
  S^T[kt, qt]  = kT_tile.T @ qT_block              (K = head dim 64; the two
                                                    heads run CONCURRENTLY via
                                                    64x128 PE row tiling --
                                                    tile_position auto-derives
                                                    from the operands' base
                                                    partitions 0 / 64)
  diag mask    = S^T += (-30000) * triu via a 128-mode matmul accumulating
                 into the same PSUM group (exp then underflows to 0; keeps
                 the DVE off the scores->exp->PV critical chain)
  P^T          = exp(S^T / 32)                     (ACT; no max subtraction --
                                                    scores are O(1), exp safe)
  outT[65,qt] += [V | ones].T @ P^T                (row 64 = softmax sums)
  attnT        = outT[0:64] * (1 / outT[64])       (DVE mul reads PSUM
                                                    directly; one batched
                                                    reciprocal + one gpsimd
                                                    partition_broadcast per
                                                    q-block covers both heads)
  y[t, f]      = attnT_tile.T @ wpT                (partial; host sums cores)

All matmul operands are float16 (host-converted); accumulation stays fp32 in
PSUM.  y is written fp16 (the host-side cross-core sum is fp32) to halve the
output DMA traffic.

Scheduling (evolved over several profiled iterations; 246us -> ~165us):
the original kernel left the PE clock-gate (HAM) oscillating between K=4/8
and 8/8 (116us throttled at 1.2 GHz) because the PE idled behind the ACT
exp every kt tile, and spent 85us of sync-queue time issuing ~140 narrow
DMA descriptors.  This version:
  * keeps the PE densely busy: ALL work outside the per-tile
    scores->exp->PV chain (QKV passes split per-output, V transposes,
    per-tile projection steps) is queued as small filler closures,
    demand-paced into the PE program between the scores of tile k and the
    PV of tile k-1 (fractional accumulator spreads the queue over the
    remaining kt-tile slots, reserving ~16 items past the last tile so the
    PE stays fed while the final normalization chain runs);
  * interleaves the two batches' q-blocks (SCHED) so filler remains
    available across the whole kernel, ending on a small q-block;
  * batches DMA into few wide descriptors (~0.6us queue cost each),
    ordered so the first matmul's dependencies land first;
  * warms the HAM clock gate with junk matmuls during the startup DMA;
  * evacuates PV to SBUF immediately (frees the single-buffered pv banks),
    and in the tail runs projection evacuations on the otherwise-idle
    scalar engine with PSUM banks borrowed from the dead scores ring.
Steady state measured at the PE streaming roofline: 215ns per 512-wide
matmul, row-tiled score pairs 3ns apart, LDWEIGHTS fully hidden.  The late
phase is ACT(exp)-bound (the big causal q-blocks cluster late); fp8 was
evaluated and rejected (q/k-only fp8 already costs 1.6e-2 rel error vs the
2e-2 budget).  Measured: ~164-166us HW exec, rel err 5.4e-4 (baseline 249us).
"""

import numpy as np

B, T, C = 2, 2048, 1024
H, D = 16, 64
NCORES = 8
HPC = H // NCORES          # heads per core = 2
BT = B * T                 # 4096 tokens total
TB = 512                   # token block (matmul moving free dim)
CK = C // 128              # 8 contraction tiles for the projections
NTB = BT // TB             # 8 token blocks
NQB = T // TB              # 4 q blocks per batch
NKT = T // 128             # 16 kt tiles per batch
SCALE = 1.0 / 32.0         # 1 / sqrt(C)
MASKNEG = -30000.0         # additive causal mask; exp((s-30000)/32) == 0

# q-block execution order: interleave the batches so that QKV passes for
# batch 1 (which must precede batch 1's attention) and projection steps
# stay available as PE filler throughout, and the ACT load is spread.
SCHED = [(0, 0), (0, 1), (0, 2), (1, 1), (0, 3), (1, 2), (1, 3), (1, 0)]


def build_program():
    """Build the single-core Bass program (same program runs on all 8 cores)."""
    from contextlib import ExitStack

    import concourse.mybir as mybir
    import concourse.tile as tile
    from concourse import bacc, library_config

    dt = mybir.dt
    F32 = dt.float32
    F16 = dt.float16

    nc = bacc.Bacc("TRN2")
    xT = nc.dram_tensor("xT", [C, BT], F16, kind="ExternalInput").ap()
    wqkvT = nc.dram_tensor("wqkvT", [C, 3 * HPC * D], F16, kind="ExternalInput").ap()
    wpT = nc.dram_tensor("wpT", [HPC * D, C], F16, kind="ExternalInput").ap()
    # consts[0] = 128x128 identity, consts[1] = additive causal mask for the
    # diagonal blocks: consts[1][i, j] = MASKNEG where j > i else 0.  Emitted
    # through the PE as lhsT with the identity as rhs, the accumulated value
    # at [kt, qt] is consts[1][qt, kt] = MASKNEG for kt > qt.
    consts = nc.dram_tensor("consts", [2, 128, 128], F16, kind="ExternalInput").ap()
    y = nc.dram_tensor("y", [BT, C], F16, kind="ExternalOutput").ap()

    with ExitStack() as ctx:
        tc = ctx.enter_context(tile.TileContext(nc))
        const = ctx.enter_context(tc.tile_pool(name="const", bufs=1))
        xpool = ctx.enter_context(tc.tile_pool(name="xload", bufs=3))
        ppool = ctx.enter_context(tc.tile_pool(name="pexp", bufs=6))
        npool = ctx.enter_context(tc.tile_pool(name="norm", bufs=2))
        ypool = ctx.enter_context(tc.tile_pool(name="yout", bufs=2))
        # PSUM budget (8 banks): s_ps 2 bufs x 2 banks = 4, pv accumulators
        # 2 x 1 bank = 2, aux ring (qkv passes / transposes / proj) 2 x 1 = 2.
        psS = ctx.enter_context(tc.tile_pool(name="psS", bufs=2, space="PSUM"))
        psPV = ctx.enter_context(tc.tile_pool(name="psPV", bufs=1, space="PSUM"))
        psAux = ctx.enter_context(tc.tile_pool(name="psAux", bufs=2, space="PSUM"))

        # ---------- constants / persistent SBUF ----------
        # batched DMAs: descriptor issue on the sync queue costs ~0.6us each,
        # so few wide descriptors beat many narrow ones
        w_sb = const.tile([128, CK, 3 * HPC * D], F16, name="w_sb")
        wqkvT_t = wqkvT.rearrange("(a p) f -> p a f", p=128)
        wp_sb = const.tile([128, C], F16, name="wp_sb")
        trident = const.tile([128, 2, 128], F16, name="trident")
        ident = trident[:, 0, :]
        trineg = trident[:, 1, :]

        def load_consts():
            nc.sync.dma_start(trident[:], consts.rearrange("c p f -> p c f"))
            nc.sync.dma_start(wp_sb[:], wpT)

        # partition_broadcast lives in the "attn" GPSIMD library; same-engine
        # FIFO order guarantees this lands before the broadcasts.
        nc.gpsimd.load_library(library_config.attn)

        # Per-batch transposed activations, heads packed on partitions
        # (h0 -> partitions 0:64, h1 -> 64:128).
        qT = [const.tile([128, T], F16, name=f"qT{b}") for b in range(B)]
        kT = [const.tile([128, T], F16, name=f"kT{b}") for b in range(B)]
        vT = [const.tile([128, T], F16, name=f"vT{b}") for b in range(B)]
        attnT = [const.tile([128, T], F16, name=f"attnT{b}") for b in range(B)]

        # PE warmup: the HAM clock gate holds the PE at 1.2 GHz until it has
        # seen ~3.4us of sustained matmul activity, and the first real matmul
        # cannot start before its DMA lands (~10us).  A few junk matmuls
        # bridge the window so the activity monitor flips to 2.4 GHz shortly
        # after the real stream begins.  The dummy memset is the FIRST DVE op
        # so the warmup starts as early as possible.
        dummy = const.tile([128, TB], F16, name="dummy")
        nc.vector.memset(dummy[:], 1.0)
        for wi in range(5):
            wps = psS.tile([128, HPC, TB], F32, name="s_ps", tag="psS")
            nc.tensor.matmul(
                wps[:, 0, :], dummy[:, 0:128], dummy[:], start=True, stop=True
            )

        # [V | ones] stationary tiles for PV: V1[:, b, kti, h, 0:64] = V
        # natural [kt, d]; column 64 = 1.0 so PV row 64 accumulates the
        # softmax sums.
        V1 = const.tile([128, B, NKT, HPC, 65], F16, name="V1")
        nc.vector.memset(V1[:, :, :, :, 64:65], 1.0)

        # ---------- QKV projection + V transpose ----------
        dest = {0: qT, 1: kT, 2: vT}
        xT_t = xT.rearrange("(ci p) t -> p ci t", p=128)
        xts = {}   # tb -> wide SBUF x tile [128, CK, TB]

        def qkv_load(tb):
            xt = xpool.tile([128, CK, TB], F16, name="xt", tag="xt")
            nc.sync.dma_start(xt[:], xT_t[:, :, tb * TB : (tb + 1) * TB])
            xts[tb] = xt

        def qkv_pass(tb, fi):
            # One output (q, k or v) of one 512-token block: 8 accumulating
            # matmuls into one PSUM bank, one DVE cast out.  ~1.8us of PE.
            b, tcol = divmod(tb, NTB // B)
            ps = psAux.tile([128, TB], F32, name="qkv_ps", tag="aux")
            for ci in range(CK):
                nc.tensor.matmul(
                    ps[:],
                    w_sb[:, ci, fi * 128 : (fi + 1) * 128],
                    xts[tb][:, ci, :],
                    start=(ci == 0),
                    stop=(ci == CK - 1),
                )
            nc.vector.tensor_copy(
                out=dest[fi][b][:, tcol * TB : (tcol + 1) * TB], in_=ps[:]
            )
            if fi == 2:
                del xts[tb]

        def v_nat(tb):
            # Both heads' V-natural tiles for this block's 4 kt tiles: one
            # 128x128 PE transpose + one strided DVE copy per kt tile.
            b, tcol = divmod(tb, NTB // B)
            for j in range(TB // 128):
                kti = tcol * (TB // 128) + j
                tr = psAux.tile([128, 128], F16, name="vtr", tag="aux")
                nc.tensor.transpose(
                    tr[:], vT[b][:, kti * 128 : (kti + 1) * 128], ident[:]
                )
                trv = tr[:].rearrange("p (h s) -> p h s", h=HPC)
                nc.vector.tensor_copy(out=V1[:, b, kti, :, 0:64], in_=trv)

        # ---------- filler queue ----------
        # Small PE work units, each tagged with the schedule position by
        # which it MUST have been emitted (data dependency of that q-block).
        # One is emitted between the scores of kt tile k and the PV of tile
        # k-1 so the PE never stalls behind the ACT exp.
        pending = []   # list of (deadline, closure); queue order respects deps
        ysbw = {}      # (b, qb, fb) -> wide fp16 staging tile for y
        pace = [0.0]   # fractional filler-emission accumulator
        tail_mode = [False]  # exp stream done -> ACT free for evacs

        def emit_pending(n=1):
            for _ in range(min(n, len(pending))):
                pending.pop(0)[1]()

        def drain_due(pos):
            # queue order carries data deps (a block's load precedes its
            # passes), so emit from the head through the LAST due item
            due = [i for i, (dl, _) in enumerate(pending) if dl <= pos]
            for _ in range((due[-1] + 1) if due else 0):
                pending.pop(0)[1]()

        # position in SCHED at which batch b's q-block tcol runs
        pos_of = {bq: i for i, bq in enumerate(SCHED)}

        # block 0 runs up front (phase 1).  DMA descriptors are ordered so
        # the FIRST matmul's dependencies (w ci 0-1, x0 ci 0-1) land first;
        # the DMA engine round-robins chunks of all in-flight descriptors,
        # so anything issued early competes with the critical path.
        xt0 = xpool.tile([128, CK, TB], F16, name="xt", tag="xt")
        nc.sync.dma_start(w_sb[:, 0:4, 0:128], wqkvT_t[:, 0:4, 0:128])
        nc.sync.dma_start(xt0[:, 0:2, :], xT_t[:, 0:2, 0:TB])
        nc.sync.dma_start(w_sb[:, 4:, 0:128], wqkvT_t[:, 4:, 0:128])
        nc.sync.dma_start(xt0[:, 2:4, :], xT_t[:, 2:4, 0:TB])
        nc.sync.dma_start(xt0[:, 4:6, :], xT_t[:, 4:6, 0:TB])
        nc.sync.dma_start(w_sb[:, :, 128:256], wqkvT_t[:, :, 128:256])
        nc.sync.dma_start(xt0[:, 6:, :], xT_t[:, 6:, 0:TB])
        nc.sync.dma_start(w_sb[:, :, 256:384], wqkvT_t[:, :, 256:384])
        xts[0] = xt0
        qkv_pass(0, 0)
        qkv_pass(0, 1)

        def deadline(tb):
            b2, tcol = divmod(tb, NTB // B)
            return min(pos_of[(b2, q2)] for q2 in range(tcol, NQB))

        blocks = sorted((deadline(tb), tb) for tb in range(1, NTB))
        # each block's x DMA is queued two block-groups (~8 filler slots)
        # ahead of its first matmul so the PE never waits on a fresh load;
        # the 3-deep xpool ring holds three blocks in flight
        groups = []
        for dl, tb in blocks:
            g = [(dl, lambda tb=tb, fi=fi: qkv_pass(tb, fi)) for fi in range(3)]
            g.append((dl, lambda tb=tb: v_nat(tb)))
            groups.append((dl, tb, g))
        qkv_load(groups[0][1])   # block-1 x DMA rides behind the startup DMAs
        load_consts()
        # v pass + V-transposes of block 0 run as the first two filler slots
        # of (0,0) (forced pops below), overlapping them with its exp stream
        pending.append((1, lambda: qkv_pass(0, 2)))
        pending.append((1, lambda: v_nat(0)))
        for gi, (dl, tb, g) in enumerate(groups):
            if gi + 1 < len(groups):
                ndl, ntb = groups[gi + 1][0], groups[gi + 1][1]
                pending.append((ndl, lambda tb=ntb: qkv_load(tb)))
            pending.extend(g)

        # ---------- attention ----------
        for pos, (b, qb) in enumerate(SCHED):
            drain_due(pos)
            nkt = (TB // 128) * qb + (TB // 128)
            pv = [
                psPV.tile([65, TB], F32, name=f"pv_ps{h}", tag=f"psPV{h}")
                for h in range(HPC)
            ]
            stages = []  # deferred PV matmuls, one kti behind the scores

            def flush(n=None):
                while stages and (n is None or len(stages) > n):
                    stages.pop(0)()

            for kti in range(nkt):
                qs = max(0, kti * 128 - qb * TB)  # local col start
                N = TB - qs
                # both heads' scores in one 2-bank PSUM tile -> one exp
                sps = psS.tile([128, HPC, TB], F32, name="s_ps", tag="psS")
                diag = kti * 128 >= qb * TB
                if diag:
                    # diagonal tile: -30000 onto kt > qt via a full-array
                    # matmul (trineg.T = the additive mask).  Emitted FIRST
                    # in the accumulation group: start=True clears the bank
                    # and sets has_written only on the masked 128 columns,
                    # so the scores accumulate there and overwrite elsewhere
                    # -- and the exp's last dependency is the scores matmul,
                    # not the mask, trimming the scores->exp chain latency.
                    for h in range(HPC):
                        nc.tensor.matmul(
                            sps[:, h, 0:128],
                            trineg[:],
                            ident[:],
                            start=True,
                            stop=False,
                        )
                for h in range(HPC):
                    hp = slice(h * 64, (h + 1) * 64)
                    nc.tensor.matmul(
                        sps[:, h, 0:N],
                        kT[b][hp, kti * 128 : (kti + 1) * 128],
                        qT[b][hp, qb * TB + qs : (qb + 1) * TB],
                        start=not diag,
                        stop=True,
                    )
                P = ppool.tile([128, HPC, TB], F16, name="Pt", tag="P")
                nc.scalar.activation(
                    P[:, :, 0:N],
                    sps[:, :, 0:N],
                    mybir.ActivationFunctionType.Exp,
                    scale=SCALE,
                )

                def pv_step(kti=kti, qs=qs, N=N, P=P):
                    for h in range(HPC):
                        nc.tensor.matmul(
                            pv[h][:, qs:TB],
                            V1[:, b, kti, h, :],
                            P[:, h, 0:N],
                            start=(kti == 0),
                            stop=(kti == nkt - 1),
                        )

                stages.append(pv_step)
                # demand-paced filler emission: spread the queue over the
                # remaining kt-tile slots (fractional accumulator).  The +10
                # phantom slots reserve ~10 items past the last kt tile, to
                # keep the PE fed while the final q-block's normalization
                # chain (DVE+gpsimd, ~6us) runs.
                phantom = 16 if pos == len(SCHED) - 1 else 4
                slots_left = sum(4 * q2 + 4 for b2, q2 in SCHED[pos:]) - kti + phantom
                pace[0] += len(pending) / max(1, slots_left)
                k = int(pace[0])
                pace[0] -= k
                force = kti < 2 or (qb >= 2 and not diag)
                emit_pending(max(k, 1 if force else 0))
                flush(1)
            flush()

            # evacuate the PV accumulators to SBUF immediately (two plain
            # copies) so the single-buffered pv banks free up ~4us earlier
            # than if the whole normalization chain read them from PSUM;
            # the normalization then works from SBUF off the critical path.
            last = pos == len(SCHED) - 1
            if last:
                tail_mode[0] = True
            pvt = [
                npool.tile([65, TB], F32, name=f"pvt{h}", tag=f"pvt{h}")
                for h in range(HPC)
            ]
            if last:
                # final q-block: per-head chains minimize the serial latency
                # from the last PV to the projection start (nothing overlaps
                # the exposed tail, so latency is wall-clock here)
                for h in range(HPC):
                    hp = slice(h * 64, (h + 1) * 64)
                    nc.vector.tensor_copy(out=pvt[h][:], in_=pv[h][:])
                    s0 = npool.tile([1, TB], F32, name="s0", tag="s0")
                    nc.vector.tensor_copy(out=s0[:], in_=pvt[h][64:65, :])
                    rt = npool.tile([1, TB], F32, name="rt", tag="rt")
                    nc.vector.reciprocal_approx_fast(rt[:], s0[:])
                    bc = npool.tile([64, TB], F32, name="bc", tag="bc")
                    nc.gpsimd.partition_broadcast(bc[:], rt[:])
                    nc.vector.tensor_mul(
                        attnT[b][hp, qb * TB : (qb + 1) * TB],
                        pvt[h][0:64, :],
                        bc[:],
                    )
            else:
                for h in range(HPC):
                    nc.vector.tensor_copy(out=pvt[h][:], in_=pv[h][:])
                s0 = npool.tile([1, HPC * TB], F32, name="s0", tag="s0")
                for h in range(HPC):
                    nc.vector.tensor_copy(
                        out=s0[:, h * TB : (h + 1) * TB], in_=pvt[h][64:65, :]
                    )
                rt = npool.tile([1, HPC * TB], F32, name="rt", tag="rt")
                nc.vector.reciprocal_approx_fast(rt[:], s0[:])
                bc = npool.tile([64, HPC * TB], F32, name="bc", tag="bc")
                nc.gpsimd.partition_broadcast(bc[:], rt[:])
                for h in range(HPC):
                    hp = slice(h * 64, (h + 1) * 64)
                    nc.vector.tensor_mul(
                        attnT[b][hp, qb * TB : (qb + 1) * TB],
                        pvt[h][0:64, :],
                        bc[:, h * TB : (h + 1) * TB],
                    )

            def proj_part(b, qb, fb, i, last):
                # one ti tile of the q-block's projection: 1 matmul + 1
                # evacuation; i == 3 also fires the batched row DMA.  In the
                # final schedule slot the exp stream is done, so half the
                # evacuations go to the otherwise-idle scalar engine.
                key = (b, qb, fb)
                if i == 0:
                    ysbw[key] = ypool.tile(
                        [128, 4, TB], F16, name="ysbw", tag=f"ysb{fb}"
                    )
                w = ysbw[key]
                ti = 4 * qb + i
                if tail_mode[0] and i % 2 == 0:
                    # the scores ring is idle after the last exp: borrow its
                    # banks so four projections can be in flight at once
                    ps = psS.tile([128, HPC, TB], F32, name="s_ps", tag="psS")[
                        :, 0, :
                    ]
                else:
                    ps = psAux.tile([128, TB], F32, name="y_ps", tag="aux")
                nc.tensor.matmul(
                    ps[:],
                    attnT[b][:, ti * 128 : (ti + 1) * 128],
                    wp_sb[:, fb * TB : (fb + 1) * TB],
                    start=True,
                    stop=True,
                )
                if tail_mode[0] and not (last and i % 2 == 1):
                    nc.scalar.copy(out=w[:, i, :], in_=ps[:])
                else:
                    nc.vector.tensor_copy(out=w[:, i, :], in_=ps[:])
                if last and i % 2 == 1:
                    # final q-block: ship each half as soon as it is ready
                    half = i // 2
                    ydst = y[
                        b * T + qb * TB + half * 256 :
                        b * T + qb * TB + (half + 1) * 256,
                        fb * TB : (fb + 1) * TB,
                    ].rearrange("(ti p) f -> p ti f", p=128)
                    nc.sync.dma_start(ydst, w[:, 2 * half : 2 * half + 2, :])
                    if i == 3:
                        del ysbw[key]
                elif i == 3:
                    ydst = y[
                        b * T + qb * TB : b * T + (qb + 1) * TB,
                        fb * TB : (fb + 1) * TB,
                    ].rearrange("(ti p) f -> p ti f", p=128)
                    nc.sync.dma_start(ydst, w[:])
                    del ysbw[key]

            for fb in range(C // TB):
                for i in range(4):
                    pending.append(
                        (len(SCHED), lambda b=b, qb=qb, fb=fb, i=i, last=last:
                         proj_part(b, qb, fb, i, last))
                    )
        while pending:
            pending.pop(0)[1]()
    nc.compile()
    return nc


def make_in_maps(x, w_attn, w_proj):
    """Host-side sharding into the per-core layouts."""
    x = np.asarray(x, dtype=np.float32)
    w_attn = np.asarray(w_attn, dtype=np.float32)
    w_proj = np.asarray(w_proj, dtype=np.float32)

    xT = np.ascontiguousarray(x.reshape(BT, C).T.astype(np.float16))
    wpT_full = np.ascontiguousarray(w_proj.T.astype(np.float16))

    in_maps = []
    for c in range(NCORES):
        rows = []
        for sec in range(3):                                # q, k, v
            for h in (HPC * c, HPC * c + 1):
                rows.extend(range(sec * C + h * D, sec * C + (h + 1) * D))
        wqkvT = np.ascontiguousarray(w_attn[rows, :].T.astype(np.float16))
        wpT = np.ascontiguousarray(
            wpT_full[c * HPC * D : (c + 1) * HPC * D, :]    # [128, 1024]
        )
        consts = np.stack(
            [
                np.eye(128, dtype=np.float16),
                (MASKNEG * np.triu(np.ones((128, 128), np.float32), k=1))
                .astype(np.float16),
            ]
        )
        in_maps.append({"xT": xT, "wqkvT": wqkvT, "wpT": wpT, "consts": consts})
    return in_maps


_PROGRAM = None


def _program():
    global _PROGRAM
    if _PROGRAM is None:
        _PROGRAM = build_program()
    return _PROGRAM


def kernel(x, w_attn, w_proj):
    from concourse.bass_utils import run_bass_kernel_spmd

    res = run_bass_kernel_spmd(
        _program(), make_in_maps(x, w_attn, w_proj), list(range(NCORES))
    )
    out = res.results[0]["y"].astype(np.float32)
    for i in range(1, NCORES):
        out += res.results[i]["y"].astype(np.float32)
    return out.reshape(B, T, C)
